# revision 1
# baseline (speedup 1.0000x reference)
"""CaptionNet Trainium2 kernel (8-core SPMD, data-parallel over batch).

Per core (batch shard Bc=32): attention-LSTM recurrence fully on-chip in a
feature-on-partition / batch-on-free layout, bf16 matmul operands with fp32
PSUM accumulation. Softmax runs without max-subtraction (logits ~N(0,0.6));
the unnormalized exp(z) column per sample is the stationary operand of the
attention einsum (enc pre-transposed to [B,F,C] on the host, streamed as the
moving operand), run as 3-way column-tiled concurrent matmuls
(tile_position=(0,{0,32,64})); 1/sum(exp) is applied per sample via the ACT
copy `scale` AP. The vocab projection is deferred and batched over all T*Bc
tokens: h-chunks stationary, vocab_W.T streamed from HBM in N-chunks, logits
written out as contiguous [T*Bc, V] rows. The host shards the batch, shifts
the teacher-forced inputs, pre-transposes and pre-casts everything.
"""

import numpy as np
import ml_dtypes

import concourse.bass as bass
import concourse.tile as tile
import concourse.mybir as mybir

BF16 = mybir.dt.bfloat16
F32 = mybir.dt.float32
AF = mybir.ActivationFunctionType
OP = mybir.AluOpType

# Problem constants (full size)
B_FULL, T_FULL, H, WV, F, C, V_FULL = 256, 20, 512, 301, 196, 512, 9871
N_CORES = 8
F_HI = 128
F_LO = F - F_HI  # 68


def _tiles(total, step=128):
    return [(i, min(step, total - i)) for i in range(0, total, step)]


def build_program(Bc=32, T=20, V=V_FULL, vchunk=512, stage=99):
    TB = Bc * T
    nc = bass.Bass()

    # ---------------- DRAM I/O (per-core) ----------------
    encT_d = nc.dram_tensor("encT", [Bc, F, C], BF16, kind="ExternalInput")
    xT_d = nc.dram_tensor("xT", [WV, TB], BF16, kind="ExternalInput")
    AxT_d = nc.dram_tensor("AxT", [WV, F], BF16, kind="ExternalInput")
    AhT_d = nc.dram_tensor("AhT", [H, F], BF16, kind="ExternalInput")
    WxT_d = nc.dram_tensor("WxT", [WV, WV], BF16, kind="ExternalInput")
    WcT_d = nc.dram_tensor("WcT", [C, WV], BF16, kind="ExternalInput")
    gateTa_d = nc.dram_tensor("gateTa", [H + 1, C], BF16, kind="ExternalInput")
    WihT_d = nc.dram_tensor("WihT", [WV, 4 * H], BF16, kind="ExternalInput")
    WhhT_d = nc.dram_tensor("WhhT", [H, 4 * H], BF16, kind="ExternalInput")
    vWT_d = nc.dram_tensor("vWT", [H, V], BF16, kind="ExternalInput")
    attnb_d = nc.dram_tensor("attn_br", [1, F], BF16, kind="ExternalInput")
    combb_d = nc.dram_tensor("comb_br", [1, WV], BF16, kind="ExternalInput")
    lstmb_d = nc.dram_tensor("lstm_bc", [128, 16, Bc], F32, kind="ExternalInput")
    eye_d = nc.dram_tensor("eye", [Bc, Bc], F32, kind="ExternalInput")
    onesc_d = nc.dram_tensor("ones_col", [F, 1], BF16, kind="ExternalInput")
    onesr_d = nc.dram_tensor("ones_row", [1, TB], BF16, kind="ExternalInput")
    out_d = nc.dram_tensor("out", [TB, V], F32, kind="ExternalOutput")

    wv_t = _tiles(WV)   # [(0,128),(128,128),(256,45)]
    h_t = _tiles(H)     # 4 x 128
    f_t = [(0, F_HI), (F_HI, F_LO)]
    NWV, NH, NF = len(wv_t), len(h_t), len(f_t)
    n_mv = _tiles(TB)   # vocab stationary chunks along T*Bc

    with tile.TileContext(nc) as tc:
        with (
            tc.tile_pool(name="w", bufs=1) as wp,
            tc.tile_pool(name="act", bufs=2) as ap,
            tc.tile_pool(name="st", bufs=2) as st,
            tc.tile_pool(name="vo", bufs=3) as vp,
            tc.tile_pool(name="ps1", bufs=1, space="PSUM") as ps1,
            tc.tile_pool(name="ps2", bufs=2, space="PSUM") as ps2,
        ):
            # ---------------- resident loads ----------------
            def load_ktiles(dram, ktiles, ncols, dt, name):
                out = []
                for ki, (k0, ks) in enumerate(ktiles):
                    tl = wp.tile([ks, ncols], dt, tag=f"{name}{ki}", name=f"{name}{ki}")
                    nc.sync.dma_start(tl[:], dram[k0 : k0 + ks, :])
                    out.append(tl)
                return out

            ones_f = load_ktiles(onesc_d, f_t, 1, BF16, "ones")
            ones1 = wp.tile([1, TB], BF16, tag="onesr", name="onesr")
            nc.sync.dma_start(ones1[:], onesr_d[:])
            eye_sb = wp.tile([Bc, Bc], F32, tag="eye", name="eye")
            nc.sync.dma_start(eye_sb[:], eye_d[:])

            enc_sb = []
            encT_r = encT_d.rearrange("b f c -> f b c")
            for fi, (f0, fs) in enumerate(f_t):
                e = wp.tile([fs, Bc, C], BF16, tag=f"enc{fi}", name=f"enc{fi}")
                nc.sync.dma_start(e[:], encT_r[f0 : f0 + fs])
                enc_sb.append(e)

            xT_sb = load_ktiles(xT_d, wv_t, TB, BF16, "xT")
            AxT_sb = load_ktiles(AxT_d, wv_t, F, BF16, "AxT")
            AhT_sb = load_ktiles(AhT_d, h_t, F, BF16, "AhT")
            WxT_sb = load_ktiles(WxT_d, wv_t, WV, BF16, "WxT")
            WcT_sb = load_ktiles(WcT_d, h_t, WV, BF16, "WcT")
            gateT_sb = load_ktiles(gateTa_d, h_t, C, BF16, "gateT")
            gateB_sb = wp.tile([1, C], BF16, tag="gateB", name="gateB")
            nc.sync.dma_start(gateB_sb[:], gateTa_d[H : H + 1, :])
            WihT_sb = load_ktiles(WihT_d, wv_t, 4 * H, BF16, "WihT")
            WhhT_sb = load_ktiles(WhhT_d, h_t, 4 * H, BF16, "WhhT")
            attnb_sb = wp.tile([1, F], BF16, tag="attnbr", name="attnbr")
            nc.sync.dma_start(attnb_sb[:], attnb_d[:])
            combb_sb = wp.tile([1, WV], BF16, tag="combbr", name="combbr")
            nc.sync.dma_start(combb_sb[:], combb_d[:])
            lstmb_sb = wp.tile([128, 16, Bc], F32, tag="lstmb", name="lstmb")
            nc.sync.dma_start(lstmb_sb[:], lstmb_d[:])

            h_all = wp.tile([128, NH, TB], BF16, tag="h_all", name="h_all")

            if stage < 1:
                return nc
            # ---------------- x-precomputes: zx = x@Ax.T ; cx = x@Wx.T ----------------
            NN = min(320, TB)

            def precompute(weights, mtiles, dst_tiles, bias_row):
                for mi, (m0, ms) in enumerate(mtiles):
                    for n0 in range(0, TB, NN):
                        nn = min(NN, TB - n0)
                        pps = ps1.tile([128, NN], F32, tag="sm", name="sm")
                        for ki in range(len(weights)):
                            nc.tensor.matmul(
                                pps[0:ms, 0:nn],
                                weights[ki][:, m0 : m0 + ms],
                                xT_sb[ki][:, n0 : n0 + nn],
                                start=(ki == 0),
                                stop=False,
                            )
                        nc.tensor.matmul(
                            pps[0:ms, 0:nn],
                            bias_row[:, m0 : m0 + ms],
                            ones1[:, n0 : n0 + nn],
                            start=False,
                            stop=True,
                        )
                        nc.vector.tensor_copy(
                            dst_tiles[mi][:, n0 : n0 + nn], pps[0:ms, 0:nn]
                        )

            zx_sb = [
                wp.tile([fs, TB], F32, tag=f"zx{fi}", name=f"zx{fi}") for fi, (f0, fs) in enumerate(f_t)
            ]
            precompute(AxT_sb, f_t, zx_sb, attnb_sb)
            cx_sb = [
                wp.tile([ms, TB], F32, tag=f"cx{mi}", name=f"cx{mi}") for mi, (m0, ms) in enumerate(wv_t)
            ]
            precompute(WxT_sb, wv_t, cx_sb, combb_sb)

            if stage < 2:
                return nc
            # ---------------- recurrence ----------------
            c_prev = None
            h_prev = None
            for t in range(T):
                tc0, tc1 = t * Bc, (t + 1) * Bc

                # attention z, h-part  (PE early)
                if t > 0:
                    zh_ps = ps1.tile([128, NF * Bc], F32, tag="sm", name="sm")
                    for mi, (m0, ms) in enumerate(f_t):
                        for ki in range(NH):
                            nc.tensor.matmul(
                                zh_ps[0:ms, mi * Bc : (mi + 1) * Bc],
                                AhT_sb[ki][:, m0 : m0 + ms],
                                h_prev[ki],
                                start=(ki == 0),
                                stop=(ki == NH - 1),
                            )

                # gamma pre-activation (independent of attention; fills PE)
                gam_ps = ps1.tile([Bc, C], F32, tag="gam", name="gam")
                if t > 0:
                    for ki in range(NH):
                        nc.tensor.matmul(
                            gam_ps[:], h_prev[ki], gateT_sb[ki][:],
                            start=(ki == 0), stop=False,
                        )
                    nc.tensor.matmul(gam_ps[:], ones1[:, 0:Bc], gateB_sb[:],
                                     start=False, stop=True)
                else:
                    nc.tensor.matmul(gam_ps[:], ones1[:, 0:Bc], gateB_sb[:],
                                     start=True, stop=True)

                # expz = exp(zx + zh + attn_b)   [f-tile, Bc] bf16
                expz = []
                for fi, (f0, fs) in enumerate(f_t):
                    ez = ap.tile([fs, Bc], BF16, tag=f"expz{fi}", name=f"expz{fi}")
                    if t > 0:
                        zs = ap.tile([fs, Bc], F32, tag=f"zsum{fi}", name=f"zsum{fi}")
                        nc.vector.tensor_tensor(
                            zs[:], zh_ps[0:fs, fi * Bc : (fi + 1) * Bc],
                            zx_sb[fi][:, tc0:tc1], op=OP.add,
                        )
                        src = zs
                    else:
                        src = zx_sb[fi][:, tc0:tc1]
                    nc.scalar.activation(ez[:], src[:], AF.Exp)
                    expz.append(ez)

                # sum over F (partitions) via ones-matmul -> [Bc, 1]
                se_ps = ps1.tile([Bc, 1], F32, tag="se", name="se")
                for fi in range(NF):
                    nc.tensor.matmul(
                        se_ps[:], expz[fi][:], ones_f[fi][:],
                        start=(fi == 0), stop=(fi == NF - 1),
                    )
                recip = ap.tile([Bc, 1], F32, tag="recip", name="recip")
                nc.vector.reciprocal(recip[:], se_ps[:])

                # einsum: ctx[b,:] = sum_f expz[b,f] * encT[b,f,:]
                # 3 concurrent column strips (psum rows 0/32/64); strip j owns
                # samples [off_j, off_j + sz_j); round r of strip j -> sample
                # off_j + r, staged in blk[32j, r, :], gathered per strip by
                # one contiguous SWDGE DMA.
                q3, rem3 = divmod(Bc, 3)
                sizes = [q3 + (1 if j < rem3 else 0) for j in range(3)]
                offs = [0, sizes[0], sizes[0] + sizes[1]]
                NR = sizes[0]
                blk = ap.tile([128, NR, C], BF16, tag="ctxblk", name="ctxblk")
                ctx_sb = ap.tile([Bc, C], BF16, tag="ctx", name="ctx")
                for r in range(NR):
                    strips = [j for j in range(3) if r < sizes[j]]
                    eps = ps2.tile([128, C], F32, tag="ein", name="ein")
                    for s in strips:
                        for fi in range(NF):
                            b = offs[s] + r
                            fs = f_t[fi][1]
                            nc.tensor.matmul(
                                eps[32 * s : 32 * s + 32, :],
                                expz[fi][:, b : b + 1].broadcast_to([fs, 32]),
                                enc_sb[fi][:, b, :],
                                start=(fi == 0),
                                stop=(fi == NF - 1),
                                tile_position=(0, 32 * s),
                            )
                    ge = 32 * strips[-1] + 32
                    nc.scalar.activation(blk[0:ge, r, :], eps[0:ge, :], AF.Copy)
                for j in range(3):
                    if sizes[j] == 0:
                        continue
                    nc.gpsimd.dma_start(
                        ctx_sb[offs[j] : offs[j] + sizes[j], :],
                        blk[32 * j : 32 * j + 1, 0 : sizes[j], :],
                    )

                # fused gate+scale: ctxg = (ctx * 1/Z) * sigmoid(gamma)
                gam_sb = ap.tile([Bc, C], F32, tag="gam_sb", name="gam_sb")
                nc.scalar.activation(gam_sb[:], gam_ps[:], AF.Sigmoid)
                ctxg = ap.tile([Bc, C], F32, tag="ctxg", name="ctxg")
                nc.vector.scalar_tensor_tensor(
                    ctxg[:], ctx_sb[:], recip[:], gam_sb[:],
                    op0=OP.mult, op1=OP.mult,
                )

                # transpose ctxg to fb layout [C-tiles, Bc], cast bf16
                ctT_ps = ps1.tile([128, NH, Bc], F32, tag="sm", name="sm")
                for j in range(NH):
                    nc.tensor.transpose(
                        ctT_ps[:, j, :], ctxg[:, j * 128 : (j + 1) * 128], eye_sb[:]
                    )
                ctxgT = ap.tile([128, NH, Bc], BF16, tag="ctxgT", name="ctxgT")
                nc.vector.tensor_copy(ctxgT[:], ctT_ps[:])

                # comb: inp = relu(cx + WcT.T@ctxgT + comb_b) -> bf16 fb
                cb_ps = ps1.tile([128, NWV, Bc], F32, tag="sm", name="sm")
                for mi, (m0, ms) in enumerate(wv_t):
                    for ki in range(NH):
                        nc.tensor.matmul(
                            cb_ps[0:ms, mi, :],
                            WcT_sb[ki][:, m0 : m0 + ms],
                            ctxgT[:, ki, :],
                            start=(ki == 0),
                            stop=(ki == NH - 1),
                        )
                csum = ap.tile([128, NWV, Bc], F32, tag="csum", name="csum")
                inp_bf = ap.tile([128, NWV, Bc], BF16, tag="inp", name="inp")
                for mi, (m0, ms) in enumerate(wv_t):
                    nc.vector.tensor_tensor(
                        csum[0:ms, mi, :], cb_ps[0:ms, mi, :],
                        cx_sb[mi][:, tc0:tc1], op=OP.add,
                    )
                    nc.scalar.activation(
                        inp_bf[0:ms, mi, :], csum[0:ms, mi, :], AF.Relu
                    )

                # LSTM gates: [128, 16, Bc] psum
                rhs_list = [
                    (inp_bf[0:ks, ki, :], WihT_sb[ki]) for ki, (k0, ks) in enumerate(wv_t)
                ]
                if t > 0:
                    rhs_list += [(h_prev[ki], WhhT_sb[ki]) for ki in range(NH)]
                g_ps = ps1.tile([128, 16, Bc], F32, tag="gates", name="gates")
                for m in range(16):
                    for j, (rhs, wt) in enumerate(rhs_list):
                        nc.tensor.matmul(
                            g_ps[:, m, :],
                            wt[:, m * 128 : (m + 1) * 128],
                            rhs,
                            start=(j == 0),
                            stop=(j == len(rhs_list) - 1),
                        )
                gsum = ap.tile([128, 16, Bc], F32, tag="gsum", name="gsum")
                nc.vector.tensor_tensor(gsum[:], g_ps[:], lstmb_sb[:], op=OP.add)
                nl = ap.tile([128, 16, Bc], F32, tag="nl", name="nl")
                for (a, b_, fn) in (
                    (0, 4, AF.Sigmoid), (4, 8, AF.Sigmoid),
                    (8, 12, AF.Tanh), (12, 16, AF.Sigmoid),
                ):
                    nc.scalar.activation(nl[:, a:b_, :], gsum[:, a:b_, :], fn)
                ig = ap.tile([128, 4, Bc], F32, tag="ig", name="ig")
                nc.vector.tensor_tensor(ig[:], nl[:, 0:4, :], nl[:, 8:12, :], op=OP.mult)
                c_new = st.tile([128, 4, Bc], F32, tag="c", name="c")
                if t > 0:
                    cf = ap.tile([128, 4, Bc], F32, tag="cf", name="cf")
                    nc.vector.tensor_tensor(cf[:], nl[:, 4:8, :], c_prev[:], op=OP.mult)
                    nc.vector.tensor_tensor(c_new[:], ig[:], cf[:], op=OP.add)
                else:
                    nc.vector.tensor_copy(c_new[:], ig[:])
                tanh_c = ap.tile([128, 4, Bc], F32, tag="tanh_c", name="tanh_c")
                nc.scalar.activation(tanh_c[:], c_new[:], AF.Tanh)
                nc.vector.tensor_tensor(
                    h_all[:, :, tc0:tc1], nl[:, 12:16, :], tanh_c[:], op=OP.mult
                )
                c_prev = c_new
                h_prev = [h_all[:, k, tc0:tc1] for k in range(NH)]

            if stage < 3:
                return nc
            # ---------------- vocab projection ----------------
            for n0 in range(0, V, vchunk):
                nn = min(vchunk, V - n0)
                vw = vp.tile([128, NH, vchunk], BF16, tag="vw", name="vw")
                for ki in range(NH):
                    nc.sync.dma_start(
                        vw[:, ki, 0:nn], vWT_d[ki * 128 : (ki + 1) * 128, n0 : n0 + nn]
                    )
                for m0, ms in n_mv:
                    vps = ps2.tile([128, C], F32, tag="ein", name="ein")
                    for ki in range(NH):
                        nc.tensor.matmul(
                            vps[0:ms, 0:nn],
                            h_all[:, ki, m0 : m0 + ms],
                            vw[:, ki, 0:nn],
                            start=(ki == 0),
                            stop=(ki == NH - 1),
                        )
                    vo = vp.tile([128, vchunk], F32, tag="vout", name="vout")
                    nc.scalar.activation(vo[0:ms, 0:nn], vps[0:ms, 0:nn], AF.Copy)
                    nc.sync.dma_start(out_d[m0 : m0 + ms, n0 : n0 + nn], vo[0:ms, 0:nn])

    _split_multi_waits(nc)
    return nc


def _split_multi_waits(nc):
    """walrus' codegen accepts at most one sync wait per engine instruction
    in this environment; hoist extra waits onto same-engine NoOps placed
    immediately before the owning instruction."""
    for fn in nc.m.functions:
        for bb in fn.blocks:
            insts = bb.instructions
            out = []
            changed = False
            for inst in insts:
                si = inst.sync_info
                if si is not None and len(si.on_wait) > 1:
                    waits = list(si.on_wait)
                    for w in waits[:-1]:
                        out.append(
                            mybir.InstNoOp(
                                name=f"{inst.name}-w{len(out)}",
                                engine=inst.engine,
                                sync_info=mybir.SyncInfo(
                                    on_wait=[w], on_update=[]
                                ),
                            )
                        )
                    inst.sync_info = mybir.SyncInfo(
                        on_wait=[waits[-1]], on_update=list(si.on_update)
                    )
                    changed = True
                out.append(inst)
            if changed:
                bb.instructions = out


# ======================= host side =======================

def _bf16(x):
    return np.ascontiguousarray(np.asarray(x, dtype=ml_dtypes.bfloat16))


def _f32(x):
    return np.ascontiguousarray(np.asarray(x, dtype=np.float32))


def prep_shared(inputs, Bc, T, V):
    """Weight-derived in_map entries (replicated across cores)."""
    attn_W = np.asarray(inputs["attn_W"], np.float32)
    comb_W = np.asarray(inputs["comb_W"], np.float32)
    gate_W = np.asarray(inputs["gate_W"], np.float32)
    sh = {
        "AxT": _bf16(attn_W[:, :WV].T),
        "AhT": _bf16(attn_W[:, WV:].T),
        "WxT": _bf16(comb_W[:, :WV].T),
        "WcT": _bf16(comb_W[:, WV:].T),
        "gateTa": _bf16(
            np.concatenate(
                [gate_W.T, np.asarray(inputs["gate_b"], np.float32)[None, :]], 0
            )
        ),
        "WihT": _bf16(np.asarray(inputs["lstm_Wih"]).T),
        "WhhT": _bf16(np.asarray(inputs["lstm_Whh"]).T),
        "vWT": _bf16(np.asarray(inputs["vocab_W"]).T[:, :V]),
        "attn_br": _bf16(np.asarray(inputs["attn_b"])[None, :]),
        "comb_br": _bf16(np.asarray(inputs["comb_b"])[None, :]),
        "eye": np.eye(Bc, dtype=np.float32),
        "ones_col": np.ones((F, 1), dtype=ml_dtypes.bfloat16),
        "ones_row": np.ones((1, T * Bc), dtype=ml_dtypes.bfloat16),
    }
    bsum = (
        np.asarray(inputs["lstm_bih"], np.float32)
        + np.asarray(inputs["lstm_bhh"], np.float32)
    )
    bb = np.ascontiguousarray(bsum.reshape(16, 128).T)  # [128, 16]
    sh["lstm_bc"] = np.ascontiguousarray(
        np.broadcast_to(bb[:, :, None], (128, 16, Bc))
    ).astype(np.float32)
    return sh


def prep_core(inputs, core, Bc, T, V):
    """Batch-sharded in_map entries for one core."""
    b0, b1 = core * Bc, (core + 1) * Bc
    enc = np.asarray(inputs["encoding"], np.float32)[b0:b1]  # [Bc, C, F]
    wv = np.asarray(inputs["wordvecs"], np.float32)[b0:b1, :T]  # [Bc, T, WV]
    x_shift = np.concatenate(
        [np.zeros((Bc, 1, WV), np.float32), wv[:, :-1, :]], axis=1
    )
    return {
        "encT": _bf16(enc.transpose(0, 2, 1)),  # [Bc, F, C]
        "xT": _bf16(x_shift.transpose(2, 1, 0).reshape(WV, T * Bc)),
    }


_PROG_CACHE = {}
LAST_RESULT = None


def kernel(**inputs):
    global LAST_RESULT
    from concourse.bass_utils import run_bass_kernel_spmd

    Bc, T, V = B_FULL // N_CORES, T_FULL, V_FULL
    key = (Bc, T, V)
    if key not in _PROG_CACHE:
        _PROG_CACHE[key] = build_program(Bc, T, V)
    nc = _PROG_CACHE[key]

    shared = prep_shared(inputs, Bc, T, V)
    in_maps = [dict(shared, **prep_core(inputs, k, Bc, T, V)) for k in range(N_CORES)]
    res = run_bass_kernel_spmd(nc, in_maps, list(range(N_CORES)))
    LAST_RESULT = res

    parts = []
    for r in res.results:
        o = np.asarray(r["out"], np.float32).reshape(T, Bc, V).transpose(1, 0, 2)
        parts.append(o)
    out = np.concatenate(parts, axis=0)
    out = out + np.asarray(inputs["vocab_b"], np.float32)[None, None, :]
    return np.ascontiguousarray(out.astype(np.float32))



# revision 6
# speedup vs baseline: 1.2132x; 1.2132x over previous
"""CaptionNet Trainium2 kernel (8-core SPMD, data-parallel over batch).

Per core (batch shard Bc=32): attention-LSTM recurrence fully on-chip in a
feature-on-partition / batch-on-free layout, bf16 matmul operands (enc fp8)
with fp32 PSUM accumulation. All sigmoids are computed as tanh(x/2) via the
identity sigmoid(x) = (1+tanh(x/2))/2 so the ACT engine never leaves the
exp_and_others table set (exp/tanh/relu/copy), eliminating the ~2.7us
ACT_TABLE_LOAD per switch. The 1/2 factors are folded into host-side weight
prescaling: the kernel carries h* = 2h and c* = 2c; AhT/gateT/WhhT/vWT are
pre-halved, and the softmax-sum ones-vector carries value 2.0 so the
reciprocal directly yields 1/(2*Z).

Softmax runs without max-subtraction (logits ~N(0,0.6)); the unnormalized
exp(z) column per sample is the stationary operand of the attention einsum
(enc pre-transposed to [B,F,C] fp8 on the host, streamed as the moving
operand), run as 3-way column-tiled concurrent matmuls. The vocab projection
weights are SBUF-resident (loaded once) and the projection is interleaved
into the recurrence: ~6 of the 100 (m-tile, v-chunk) work items are emitted
after each timestep, filling PE gaps left by the serial recurrence chain.
Logits are written to HBM as bf16; the host casts to f32 and adds vocab_b.
"""

import numpy as np
import ml_dtypes

import concourse.bass as bass
import concourse.tile as tile
import concourse.mybir as mybir

BF16 = mybir.dt.bfloat16
FP8 = mybir.dt.float8e4
F32 = mybir.dt.float32
AF = mybir.ActivationFunctionType
OP = mybir.AluOpType

# Problem constants (full size)
B_FULL, T_FULL, H, WV, F, C, V_FULL = 256, 20, 512, 301, 196, 512, 9871
N_CORES = 8
F_HI = 128
F_LO = F - F_HI  # 68


def _tiles(total, step=128):
    return [(i, min(step, total - i)) for i in range(0, total, step)]


def build_program(Bc=32, T=20, V=V_FULL, vchunk=512, stage=99):
    TB = Bc * T
    nc = bass.Bass()

    # ---------------- DRAM I/O (per-core) ----------------
    encT_d = nc.dram_tensor("encT", [Bc, F, C], FP8, kind="ExternalInput")
    xT_d = nc.dram_tensor("xT", [WV, TB], BF16, kind="ExternalInput")
    AxT_d = nc.dram_tensor("AxT", [WV, F], BF16, kind="ExternalInput")
    AhT_d = nc.dram_tensor("AhT", [H, F], BF16, kind="ExternalInput")
    WxT_d = nc.dram_tensor("WxT", [WV, WV], BF16, kind="ExternalInput")
    WcT_d = nc.dram_tensor("WcT", [C, WV], BF16, kind="ExternalInput")
    gateTa_d = nc.dram_tensor("gateTa", [H + 1, C], BF16, kind="ExternalInput")
    WihT_d = nc.dram_tensor("WihT", [WV, 4 * H], BF16, kind="ExternalInput")
    WhhT_d = nc.dram_tensor("WhhT", [H, 4 * H], BF16, kind="ExternalInput")
    vWT_d = nc.dram_tensor("vWT", [H, V], BF16, kind="ExternalInput")
    attnb_d = nc.dram_tensor("attn_br", [1, F], BF16, kind="ExternalInput")
    combb_d = nc.dram_tensor("comb_br", [1, WV], BF16, kind="ExternalInput")
    lstmb_d = nc.dram_tensor("lstm_bc", [128, 16, Bc], F32, kind="ExternalInput")
    eye_d = nc.dram_tensor("eye", [Bc, Bc], BF16, kind="ExternalInput")
    twosc_d = nc.dram_tensor("twos_col", [F, 1], BF16, kind="ExternalInput")
    onesr_d = nc.dram_tensor("ones_row", [1, TB], BF16, kind="ExternalInput")
    out_d = nc.dram_tensor("out", [TB, V], BF16, kind="ExternalOutput")

    wv_t = _tiles(WV)   # [(0,128),(128,128),(256,45)]
    h_t = _tiles(H)     # 4 x 128
    f_t = [(0, F_HI), (F_HI, F_LO)]
    NWV, NH, NF = len(wv_t), len(h_t), len(f_t)
    n_mv = _tiles(TB)   # vocab m-tiles along T*Bc (5 x 128)

    # vocab work queue: (m-tile index, chunk start) in m-major order
    vq = [(mi, n0) for mi, (m0, ms) in enumerate(n_mv)
          for n0 in range(0, V, vchunk)]
    NCH = (V + vchunk - 1) // vchunk  # 20 chunks per m-tile

    with tile.TileContext(nc) as tc:
        with (
            tc.tile_pool(name="w", bufs=1) as wp,
            tc.tile_pool(name="act", bufs=2) as ap,
            tc.tile_pool(name="big", bufs=1) as bp,
            tc.tile_pool(name="st", bufs=2) as st,
            tc.tile_pool(name="vo", bufs=3) as vp,
            tc.tile_pool(name="ps1", bufs=1, space="PSUM") as ps1,
            tc.tile_pool(name="ps2", bufs=2, space="PSUM") as ps2,
            tc.tile_pool(name="psv", bufs=2, space="PSUM") as psv,
        ):
            # ---------------- resident loads (step-0 critical first) ----
            def load_ktiles(dram, ktiles, ncols, dt, name):
                out = []
                for ki, (k0, ks) in enumerate(ktiles):
                    tl = wp.tile([ks, ncols], dt, tag=f"{name}{ki}", name=f"{name}{ki}")
                    nc.sync.dma_start(tl[:], dram[k0 : k0 + ks, :])
                    out.append(tl)
                return out

            xT_sb = load_ktiles(xT_d, wv_t, TB, BF16, "xT")
            AxT_sb = load_ktiles(AxT_d, wv_t, F, BF16, "AxT")
            WxT_sb = load_ktiles(WxT_d, wv_t, WV, BF16, "WxT")
            attnb_sb = wp.tile([1, F], BF16, tag="attnbr", name="attnbr")
            nc.sync.dma_start(attnb_sb[:], attnb_d[:])
            combb_sb = wp.tile([1, WV], BF16, tag="combbr", name="combbr")
            nc.sync.dma_start(combb_sb[:], combb_d[:])
            ones1 = wp.tile([1, TB], BF16, tag="onesr", name="onesr")
            nc.sync.dma_start(ones1[:], onesr_d[:])
            twos_f = load_ktiles(twosc_d, f_t, 1, BF16, "twos")
            eye_sb = wp.tile([Bc, Bc], BF16, tag="eye", name="eye")
            nc.sync.dma_start(eye_sb[:], eye_d[:])

            AhT_sb = load_ktiles(AhT_d, h_t, F, BF16, "AhT")
            WcT_sb = load_ktiles(WcT_d, h_t, WV, BF16, "WcT")
            gateT_sb = load_ktiles(gateTa_d, h_t, C, BF16, "gateT")
            gateB_sb = wp.tile([1, C], BF16, tag="gateB", name="gateB")
            nc.sync.dma_start(gateB_sb[:], gateTa_d[H : H + 1, :])
            WihT_sb = load_ktiles(WihT_d, wv_t, 4 * H, BF16, "WihT")
            WhhT_sb = load_ktiles(WhhT_d, h_t, 4 * H, BF16, "WhhT")
            lstmb_sb = wp.tile([128, 16, Bc], F32, tag="lstmb", name="lstmb")
            nc.sync.dma_start(lstmb_sb[:], lstmb_d[:])

            enc_sb = []
            encT_r = encT_d.rearrange("b f c -> f b c")
            for fi, (f0, fs) in enumerate(f_t):
                e = wp.tile([fs, Bc, C], FP8, tag=f"enc{fi}", name=f"enc{fi}")
                nc.sync.dma_start(e[:], encT_r[f0 : f0 + fs])
                enc_sb.append(e)

            # vocab weights resident (needed first at end of step 3)
            vw = wp.tile([128, NH, V], BF16, tag="vw", name="vw")
            for ki in range(NH):
                nc.sync.dma_start(vw[:, ki, :], vWT_d[ki * 128 : (ki + 1) * 128, :])

            # h* per 128-token segment (disjoint tiles so interleaved vocab
            # reads never WAR-block the recurrence's h writes)
            h_seg = [
                wp.tile([128, NH, 4 * Bc], BF16, tag=f"h{mi}", name=f"h{mi}")
                for mi in range(len(n_mv))
            ]

            if stage < 1:
                return nc
            # ---------------- x-precomputes: zx = x@Ax.T ; cx = x@Wx.T ----
            NN = min(320, TB)

            def precompute(weights, mtiles, dst_tiles, bias_row):
                for mi, (m0, ms) in enumerate(mtiles):
                    for n0 in range(0, TB, NN):
                        nn = min(NN, TB - n0)
                        pps = ps1.tile([128, NN], F32, tag="sm", name="sm")
                        for ki in range(len(weights)):
                            nc.tensor.matmul(
                                pps[0:ms, 0:nn],
                                weights[ki][:, m0 : m0 + ms],
                                xT_sb[ki][:, n0 : n0 + nn],
                                start=(ki == 0),
                                stop=False,
                            )
                        nc.tensor.matmul(
                            pps[0:ms, 0:nn],
                            bias_row[:, m0 : m0 + ms],
                            ones1[:, n0 : n0 + nn],
                            start=False,
                            stop=True,
                        )
                        nc.vector.tensor_copy(
                            dst_tiles[mi][:, n0 : n0 + nn], pps[0:ms, 0:nn]
                        )

            zx_sb = [
                wp.tile([fs, TB], BF16, tag=f"zx{fi}", name=f"zx{fi}")
                for fi, (f0, fs) in enumerate(f_t)
            ]
            precompute(AxT_sb, f_t, zx_sb, attnb_sb)
            cx_sb = [
                wp.tile([ms, TB], BF16, tag=f"cx{mi}", name=f"cx{mi}")
                for mi, (m0, ms) in enumerate(wv_t)
            ]
            precompute(WxT_sb, wv_t, cx_sb, combb_sb)

            if stage < 2:
                return nc

            # vocab chunk emitter (PE gap filler)
            vq_pos = [0]

            def emit_vocab(upto):
                while vq_pos[0] < upto:
                    mi, n0 = vq[vq_pos[0]]
                    m0, ms = n_mv[mi]
                    nn = min(vchunk, V - n0)
                    vps = psv.tile([128, vchunk], F32, tag="vps", name="vps")
                    for ki in range(NH):
                        nc.tensor.matmul(
                            vps[0:ms, 0:nn],
                            h_seg[mi][:, ki, 0:ms],
                            vw[:, ki, n0 : n0 + nn],
                            start=(ki == 0),
                            stop=(ki == NH - 1),
                        )
                    vo = vp.tile([128, vchunk], BF16, tag="vout", name="vout")
                    nc.any.tensor_copy(vo[0:ms, 0:nn], vps[0:ms, 0:nn])
                    nc.sync.dma_start(out_d[m0 : m0 + ms, n0 : n0 + nn], vo[0:ms, 0:nn])
                    vq_pos[0] += 1

            # ---------------- recurrence ----------------
            c_prev = None
            h_prev = None
            for t in range(T):
                tc0, tc1 = t * Bc, (t + 1) * Bc
                seg, sc0, sc1 = t // 4, (t % 4) * Bc, (t % 4 + 1) * Bc

                # attention z, h-part  (PE early)
                if t > 0:
                    zh_ps = ps1.tile([128, NF * Bc], F32, tag="sm", name="sm")
                    for mi, (m0, ms) in enumerate(f_t):
                        for ki in range(NH):
                            nc.tensor.matmul(
                                zh_ps[0:ms, mi * Bc : (mi + 1) * Bc],
                                AhT_sb[ki][:, m0 : m0 + ms],
                                h_prev[ki],
                                start=(ki == 0),
                                stop=(ki == NH - 1),
                            )

                # gamma pre-activation (independent of attention; fills PE)
                gam_ps = ps1.tile([Bc, C], F32, tag="gam", name="gam")
                if t > 0:
                    for ki in range(NH):
                        nc.tensor.matmul(
                            gam_ps[:], h_prev[ki], gateT_sb[ki][:],
                            start=(ki == 0), stop=False,
                        )
                    nc.tensor.matmul(gam_ps[:], ones1[:, 0:Bc], gateB_sb[:],
                                     start=False, stop=True)
                else:
                    nc.tensor.matmul(gam_ps[:], ones1[:, 0:Bc], gateB_sb[:],
                                     start=True, stop=True)

                # expz = exp(zx + zh)   [f-tile, Bc] bf16
                expz = []
                for fi, (f0, fs) in enumerate(f_t):
                    ez = ap.tile([fs, Bc], BF16, tag=f"expz{fi}", name=f"expz{fi}")
                    if t > 0:
                        zs = ap.tile([fs, Bc], BF16, tag=f"zsum{fi}", name=f"zsum{fi}")
                        nc.vector.tensor_tensor(
                            zs[:], zh_ps[0:fs, fi * Bc : (fi + 1) * Bc],
                            zx_sb[fi][:, tc0:tc1], op=OP.add,
                        )
                        src = zs
                    else:
                        src = zx_sb[fi][:, tc0:tc1]
                    nc.scalar.activation(ez[:], src[:], AF.Exp)
                    expz.append(ez)

                # 2*sum over F (partitions) via twos-matmul -> [Bc, 1]
                se_ps = ps1.tile([Bc, 1], F32, tag="se", name="se")
                for fi in range(NF):
                    nc.tensor.matmul(
                        se_ps[:], expz[fi][:], twos_f[fi][:],
                        start=(fi == 0), stop=(fi == NF - 1),
                    )
                recip2 = ap.tile([Bc, 1], F32, tag="recip", name="recip")
                nc.vector.reciprocal(recip2[:], se_ps[:])

                # gamma: gam_t = tanh(0.5*gamma_pre); gp1 = 1 + gam_t
                gam_t = ap.tile([Bc, C], BF16, tag="gam_t", name="gam_t")
                nc.scalar.activation(gam_t[:], gam_ps[:], AF.Tanh, scale=0.5)
                gp1 = ap.tile([Bc, C], BF16, tag="gp1", name="gp1")
                nc.vector.tensor_scalar_add(gp1[:], gam_t[:], 1.0)

                # einsum: ctx[b,:] = sum_f expz[b,f] * encT[b,f,:]
                # 3 concurrent column strips (psum col groups 0/32/64)
                q3, rem3 = divmod(Bc, 3)
                sizes = [q3 + (1 if j < rem3 else 0) for j in range(3)]
                offs = [0, sizes[0], sizes[0] + sizes[1]]
                NR = sizes[0]
                blk = bp.tile([128, NR, C], BF16, tag="ctxblk", name="ctxblk")
                ctx_sb = ap.tile([Bc, C], BF16, tag="ctx", name="ctx")
                for r in range(NR):
                    strips = [j for j in range(3) if r < sizes[j]]
                    eps = ps2.tile([128, C], F32, tag="ein", name="ein")
                    for s in strips:
                        for fi in range(NF):
                            b = offs[s] + r
                            fs = f_t[fi][1]
                            nc.tensor.matmul(
                                eps[32 * s : 32 * s + 32, :],
                                expz[fi][:, b : b + 1].broadcast_to([fs, 32]),
                                enc_sb[fi][:, b, :],
                                start=(fi == 0),
                                stop=(fi == NF - 1),
                                tile_position=(0, 32 * s),
                            )
                    ge = 32 * strips[-1] + 32
                    if r % 2 == 0:
                        nc.vector.tensor_copy(blk[0:ge, r, :], eps[0:ge, :])
                    else:
                        nc.scalar.activation(blk[0:ge, r, :], eps[0:ge, :], AF.Copy)
                for j in range(3):
                    nc.gpsimd.dma_start(
                        ctx_sb[offs[j] : offs[j] + sizes[j], :],
                        blk[32 * j : 32 * j + 1, 0 : sizes[j], :],
                    )

                # ctxg = (ctx * 1/(2Z)) * (1 + gam_t)  [== ctx_raw/Z * sigmoid]
                ctxg = ap.tile([Bc, C], BF16, tag="ctxg", name="ctxg")
                nc.vector.scalar_tensor_tensor(
                    ctxg[:], ctx_sb[:], recip2[:], gp1[:],
                    op0=OP.mult, op1=OP.mult,
                )

                # transpose ctxg to fb layout [C-tiles, Bc] (bf16 PE transpose)
                ctT_ps = ps1.tile([128, NH, Bc], BF16, tag="se", name="ctT")
                for j in range(NH):
                    nc.tensor.transpose(
                        ctT_ps[:, j, :], ctxg[:, j * 128 : (j + 1) * 128], eye_sb[:]
                    )
                ctxgT = ap.tile([128, NH, Bc], BF16, tag="ctxgT", name="ctxgT")
                nc.vector.tensor_copy(ctxgT[:], ctT_ps[:])

                # comb: inp = relu(cx + WcT.T@ctxgT) -> bf16 fb
                cb_ps = ps1.tile([128, NWV, Bc], F32, tag="sm", name="sm")
                for mi, (m0, ms) in enumerate(wv_t):
                    for ki in range(NH):
                        nc.tensor.matmul(
                            cb_ps[0:ms, mi, :],
                            WcT_sb[ki][:, m0 : m0 + ms],
                            ctxgT[:, ki, :],
                            start=(ki == 0),
                            stop=(ki == NH - 1),
                        )
                csum = ap.tile([128, NWV, Bc], F32, tag="csum", name="csum")
                inp_bf = ap.tile([128, NWV, Bc], BF16, tag="inp", name="inp")
                for mi, (m0, ms) in enumerate(wv_t):
                    nc.vector.tensor_tensor(
                        csum[0:ms, mi, :], cb_ps[0:ms, mi, :],
                        cx_sb[mi][:, tc0:tc1], op=OP.add,
                    )
                    nc.scalar.activation(
                        inp_bf[0:ms, mi, :], csum[0:ms, mi, :], AF.Relu
                    )

                # LSTM gates: [128, 16, Bc] psum (gate m-major: i f g o x4)
                rhs_list = [
                    (inp_bf[0:ks, ki, :], WihT_sb[ki]) for ki, (k0, ks) in enumerate(wv_t)
                ]
                if t > 0:
                    rhs_list += [(h_prev[ki], WhhT_sb[ki]) for ki in range(NH)]
                g_ps = ps1.tile([128, 16, Bc], F32, tag="gates", name="gates")
                for m in range(16):
                    for j, (rhs, wt) in enumerate(rhs_list):
                        nc.tensor.matmul(
                            g_ps[:, m, :],
                            wt[:, m * 128 : (m + 1) * 128],
                            rhs,
                            start=(j == 0),
                            stop=(j == len(rhs_list) - 1),
                        )
                gsum = ap.tile([128, 16, Bc], F32, tag="gsum", name="gsum")
                nc.vector.tensor_tensor(gsum[:], g_ps[:], lstmb_sb[:], op=OP.add)
                # t_i,t_f = tanh(gates/2) rows 0:8 ; g~ = tanh rows 8:12 ;
                # t_o = tanh(gates/2) rows 12:16  (sigmoid == (1+tanh(x/2))/2)
                nl = ap.tile([128, 16, Bc], F32, tag="nl", name="nl")
                nc.scalar.activation(nl[:, 0:8, :], gsum[:, 0:8, :], AF.Tanh, scale=0.5)
                nc.scalar.activation(nl[:, 8:12, :], gsum[:, 8:12, :], AF.Tanh)
                nc.scalar.activation(nl[:, 12:16, :], gsum[:, 12:16, :], AF.Tanh,
                                     scale=0.5)
                # c* = 2c ;  c*_new = 0.5*(1+t_f)*c*_prev + (1+t_i)*g~
                ig2 = ap.tile([128, 4, Bc], F32, tag="ig2", name="ig2")
                nc.vector.scalar_tensor_tensor(
                    ig2[:], nl[:, 0:4, :], 1.0, nl[:, 8:12, :],
                    op0=OP.add, op1=OP.mult,
                )
                c_new = st.tile([128, 4, Bc], F32, tag="c", name="c")
                if t > 0:
                    fc2 = ap.tile([128, 4, Bc], F32, tag="fc2", name="fc2")
                    nc.vector.scalar_tensor_tensor(
                        fc2[:], nl[:, 4:8, :], 1.0, c_prev[:],
                        op0=OP.add, op1=OP.mult,
                    )
                    nc.vector.scalar_tensor_tensor(
                        c_new[:], fc2[:], 0.5, ig2[:],
                        op0=OP.mult, op1=OP.add,
                    )
                else:
                    nc.vector.tensor_copy(c_new[:], ig2[:])
                # tanh(c) = tanh(c*/2) ;  h* = 2h = (1+t_o)*tanh(c)
                tanh_c = ap.tile([128, 4, Bc], F32, tag="tanh_c", name="tanh_c")
                nc.scalar.activation(tanh_c[:], c_new[:], AF.Tanh, scale=0.5)
                nc.vector.scalar_tensor_tensor(
                    h_seg[seg][:, :, sc0:sc1], nl[:, 12:16, :], 1.0, tanh_c[:],
                    op0=OP.add, op1=OP.mult,
                )
                c_prev = c_new
                h_prev = [h_seg[seg][:, k, sc0:sc1] for k in range(NH)]

                # vocab gap-filler: ~6 chunks/step once tokens are ready
                if stage >= 3 and t >= 4:
                    upto = min(6 * (t - 3), NCH * ((t + 1) // 4))
                    emit_vocab(min(upto, len(vq)))

            if stage < 3:
                return nc
            # ---------------- vocab tail ----------------
            emit_vocab(len(vq))

    _split_multi_waits(nc)
    return nc


def _split_multi_waits(nc):
    """walrus' codegen accepts at most one sync wait per engine instruction
    in this environment; hoist extra waits onto same-engine NoOps placed
    immediately before the owning instruction."""
    for fn in nc.m.functions:
        for bb in fn.blocks:
            insts = bb.instructions
            out = []
            changed = False
            for inst in insts:
                si = inst.sync_info
                if si is not None and len(si.on_wait) > 1:
                    waits = list(si.on_wait)
                    for w in waits[:-1]:
                        out.append(
                            mybir.InstNoOp(
                                name=f"{inst.name}-w{len(out)}",
                                engine=inst.engine,
                                sync_info=mybir.SyncInfo(
                                    on_wait=[w], on_update=[]
                                ),
                            )
                        )
                    inst.sync_info = mybir.SyncInfo(
                        on_wait=[waits[-1]], on_update=list(si.on_update)
                    )
                    changed = True
                out.append(inst)
            if changed:
                bb.instructions = out


# ======================= host side =======================

def _bf16(x):
    return np.ascontiguousarray(np.asarray(x, dtype=ml_dtypes.bfloat16))


def prep_shared(inputs, Bc, T, V):
    """Weight-derived in_map entries (replicated across cores).

    AhT/gateT/WhhT/vWT are pre-halved: the kernel's recurrent state is
    h* = 2h, so W @ (h*/2) == (W/2) @ h*."""
    attn_W = np.asarray(inputs["attn_W"], np.float32)
    comb_W = np.asarray(inputs["comb_W"], np.float32)
    gate_W = np.asarray(inputs["gate_W"], np.float32)
    sh = {
        "AxT": _bf16(attn_W[:, :WV].T),
        "AhT": _bf16(0.5 * attn_W[:, WV:].T),
        "WxT": _bf16(comb_W[:, :WV].T),
        "WcT": _bf16(comb_W[:, WV:].T),
        "gateTa": _bf16(
            np.concatenate(
                [0.5 * gate_W.T, np.asarray(inputs["gate_b"], np.float32)[None, :]], 0
            )
        ),
        "WihT": _bf16(np.asarray(inputs["lstm_Wih"]).T),
        "WhhT": _bf16(0.5 * np.asarray(inputs["lstm_Whh"]).T),
        "vWT": _bf16(0.5 * np.asarray(inputs["vocab_W"]).T[:, :V]),
        "attn_br": _bf16(np.asarray(inputs["attn_b"])[None, :]),
        "comb_br": _bf16(np.asarray(inputs["comb_b"])[None, :]),
        "eye": np.eye(Bc, dtype=ml_dtypes.bfloat16),
        "twos_col": np.full((F, 1), 2.0, dtype=ml_dtypes.bfloat16),
        "ones_row": np.ones((1, T * Bc), dtype=ml_dtypes.bfloat16),
    }
    bsum = (
        np.asarray(inputs["lstm_bih"], np.float32)
        + np.asarray(inputs["lstm_bhh"], np.float32)
    )
    bb = np.ascontiguousarray(bsum.reshape(16, 128).T)  # [128, 16]
    sh["lstm_bc"] = np.ascontiguousarray(
        np.broadcast_to(bb[:, :, None], (128, 16, Bc))
    ).astype(np.float32)
    return sh


def prep_core(inputs, core, Bc, T, V):
    """Batch-sharded in_map entries for one core."""
    b0, b1 = core * Bc, (core + 1) * Bc
    enc = np.asarray(inputs["encoding"], np.float32)[b0:b1]  # [Bc, C, F]
    wv = np.asarray(inputs["wordvecs"], np.float32)[b0:b1, :T]  # [Bc, T, WV]
    x_shift = np.concatenate(
        [np.zeros((Bc, 1, WV), np.float32), wv[:, :-1, :]], axis=1
    )
    return {
        "encT": np.ascontiguousarray(
            enc.transpose(0, 2, 1).astype(ml_dtypes.float8_e4m3)
        ),  # [Bc, F, C]
        "xT": _bf16(x_shift.transpose(2, 1, 0).reshape(WV, T * Bc)),
    }


_PROG_CACHE = {}
LAST_RESULT = None


def kernel(**inputs):
    global LAST_RESULT
    from concourse.bass_utils import run_bass_kernel_spmd

    Bc, T, V = B_FULL // N_CORES, T_FULL, V_FULL
    key = (Bc, T, V)
    if key not in _PROG_CACHE:
        _PROG_CACHE[key] = build_program(Bc, T, V)
    nc = _PROG_CACHE[key]

    shared = prep_shared(inputs, Bc, T, V)
    in_maps = [dict(shared, **prep_core(inputs, k, Bc, T, V)) for k in range(N_CORES)]
    res = run_bass_kernel_spmd(nc, in_maps, list(range(N_CORES)))
    LAST_RESULT = res

    parts = []
    for r in res.results:
        o = np.asarray(r["out"], np.float32).reshape(T, Bc, V).transpose(1, 0, 2)
        parts.append(o)
    out = np.concatenate(parts, axis=0)
    out = out + np.asarray(inputs["vocab_b"], np.float32)[None, None, :]
    return np.ascontiguousarray(out.astype(np.float32))
